# revision 8
# baseline (speedup 1.0000x reference)
"""Trainium2 Bass kernel for nn_MoE_3616362463841 (moe_routing).

Math (derived from the reference):
  B=4096, E=8, P=256, K=16, F=64.  xp = x.reshape(B, P, K)
  s[b,k]   = sum_p xp[b,p,k]                      (only place the 64 MiB of x is read)
  h[b,e]   = sum_k s[b,k]*gw[e,k] + P*gb[e] + noise[b]
  pi_val   = max_e h ; mask = one_hot(argmax_e h)
  t0[b,e]  = sum_k s[b,k]*w0[e,k] + P*sum_{f<F} eb[e,f]   (w0 = sum_{f<F} ew[e,f,:])
  t1[b,e]  = likewise for the second half of ew/eb
  out[b,0] = pi_val[b] * t0[b, argmax] ; out[b,1] = pi_val[b] * t1[b, argmax]
  loss     = E * sum_e mean_b(h[:,e]) * mean_b(mask[:,e])

Sharding: batch-parallel over 8 cores, 512 tokens/core, no collectives.
Per core: 4 token-tiles of [128, 4096]; contiguous 2 MiB DMA -> DVE strided
reduce to s -> PE transpose -> one PE matmul against W_aug[18,24]
(cols: h | t0 | t1; row 16 = biases via constant-1 input row, row 17
feeds noise into the h columns) -> small DVE ops for mask/output ->
PE matmul with a ones vector for per-core loss partial sums.
"""

import numpy as np

B, E, P, K, F = 4096, 8, 256, 16, 64
NCORES = 8
TOK = B // NCORES          # tokens per core (512)
NT = TOK // 128            # token tiles per core (4)
L = P * K                  # 4096

_cache = {}


def _build():
    import concourse.bacc as bacc
    import concourse.mybir as mybir
    import concourse.tile as tile

    f32 = mybir.dt.float32
    nc = bacc.Bacc()

    x_d = nc.declare_dram_parameter("x", [TOK, L], f32, isOutput=False)
    noise_d = nc.declare_dram_parameter("noise", [TOK, 1], f32, isOutput=False)
    gw_d = nc.declare_dram_parameter("gw", [E, K], f32, isOutput=False)
    gb_d = nc.declare_dram_parameter("gb", [E], f32, isOutput=False)
    ew_d = nc.declare_dram_parameter("ew", [E, 2 * F, K], f32, isOutput=False)
    eb_d = nc.declare_dram_parameter("eb", [E, 2 * F], f32, isOutput=False)
    ident_d = nc.declare_dram_parameter("ident", [128, 128], f32, isOutput=False)
    perm_d = nc.declare_dram_parameter("perm16", [16, 16], f32, isOutput=False)
    wrow17_d = nc.declare_dram_parameter("wrow17", [1, 24], f32, isOutput=False)
    out_d = nc.declare_dram_parameter("out", [TOK, 2], f32, isOutput=True)
    mask_d = nc.declare_dram_parameter("masko", [TOK, E], f32, isOutput=True)
    lsum_d = nc.declare_dram_parameter("lsums", [16, 1], f32, isOutput=True)

    with tile.TileContext(nc) as tc:
        with (
            tc.tile_pool(name="const", bufs=1) as cpool,
            tc.tile_pool(name="xin", bufs=3) as xpool,
            tc.tile_pool(name="work", bufs=2) as wpool,
            tc.tile_pool(name="psA", bufs=2, space="PSUM") as psA,
            tc.tile_pool(name="psB", bufs=2, space="PSUM") as psB,
            tc.tile_pool(name="psC", bufs=1, space="PSUM") as psC,
        ):
            # ---- constants / weight prep (once) ----
            ident = cpool.tile([128, 128], f32)
            nc.scalar.dma_start(ident[:], ident_d[:])
            perm16 = cpool.tile([16, 16], f32)
            nc.scalar.dma_start(perm16[:], perm_d[:])
            ones_col = cpool.tile([128, 1], f32)
            nc.gpsimd.memset(ones_col[:], 1.0)

            # gw_a: cols 0:16 = gw[e,k], col 16 = gb[e]  (bias rides as an
            # extra lhsT free column -> lands in psW row 16 via the PE).
            # Padded to 18 cols: fp32 matmul faults with an odd stationary
            # free size (M=17 crashed the exec unit; M=16/18 are fine).
            gw_a = cpool.tile([E, K + 2], f32)
            nc.scalar.dma_start(gw_a[:, 0:K], gw_d[:])
            nc.scalar.dma_start(gw_a[:, K:K + 1], gb_d[:].unsqueeze(1))
            nc.gpsimd.memset(gw_a[:, K + 1:K + 2], 0.0)
            # ew as [(e,h), f*k] : partition = expert/half pair
            ew_sb = cpool.tile([16, F * K], f32)
            nc.scalar.dma_start(
                ew_sb[:], ew_d[:].rearrange("e (h f) k -> (e h) (f k)", h=2)
            )
            # eb as [(e,h), f]  (contiguous view of [E, 2F])
            eb_sb = cpool.tile([16, F], f32)
            nc.scalar.dma_start(eb_sb[:], eb_d[:].rearrange("e (h f) -> (e h) f", h=2))

            # w01a[(e,h), 0:16] = sum_f ew ; w01a[(e,h), 16] = sum_f eb
            w01a = cpool.tile([16, K + 2], f32)
            nc.vector.reduce_sum(
                w01a[:, 0:K], ew_sb[:].rearrange("p (f k) -> p k f", k=K), axis=mybir.AxisListType.X
            )
            nc.vector.reduce_sum(w01a[:, K:K + 1], eb_sb[:], axis=mybir.AxisListType.X)
            nc.gpsimd.memset(w01a[:, K + 1:K + 2], 0.0)

            # psW rows 0:16 = weights (k-major), row 16 = raw biases, row 17 = 0
            psW = psC.tile([K + 2, 24], f32)
            nc.tensor.matmul(psW[:, 0:8], gw_a[:], ident[0:8, 0:8], start=True, stop=True)
            nc.tensor.matmul(psW[:, 8:24], w01a[:], perm16[:], start=True, stop=True)

            W = cpool.tile([18, 24], f32)
            nc.vector.tensor_copy(W[:], psW[:])
            # row 17: noise feed for the h columns ([1]*8 + [0]*16, from host)
            # (the P scale for row 16 rides on s_tile col 16 = 256.0)
            nc.scalar.dma_start(W[17:18, :], wrow17_d[:])

            # s_tile cols: 0:16 = s, 16 = const P (bias scale), 17 = noise
            s_tile = cpool.tile([128, 18], f32)
            nc.gpsimd.memset(s_tile[:, 16:17], float(P))

            ploss = psC.tile([16, NT], f32)

            # ---- main loop over token tiles ----
            for t in range(NT):
                xt = xpool.tile([128, L], f32)
                nc.sync.dma_start(xt[:], x_d[t * 128:(t + 1) * 128, :])
                nc.scalar.dma_start(s_tile[:, 17:18], noise_d[t * 128:(t + 1) * 128, :])

                nc.vector.reduce_sum(
                    s_tile[:, 0:16],
                    xt[:].rearrange("b (p k) -> b k p", k=K),
                    axis=mybir.AxisListType.X,
                )

                psT = psA.tile([18, 128], f32)
                nc.tensor.transpose(psT[:], s_tile[:], ident[:])
                sT = wpool.tile([18, 128], f32)
                nc.scalar.copy(sT[:], psT[:])

                r = psB.tile([128, 24], f32)
                nc.tensor.matmul(r[:], sT[:], W[:], start=True, stop=True)

                hm = wpool.tile([128, 16], f32)
                nc.scalar.copy(hm[:, 0:8], r[:, 0:8])
                pi = wpool.tile([128, 1], f32)
                nc.vector.reduce_max(pi[:], r[:, 0:8], axis=mybir.AxisListType.X)
                nc.vector.tensor_scalar(
                    hm[:, 8:16], r[:, 0:8], pi[:], None, op0=mybir.AluOpType.is_equal
                )

                mt = wpool.tile([128, 16], f32)
                nc.vector.tensor_tensor(
                    mt[:].rearrange("b (j e) -> b j e", j=2),
                    hm[:, 8:16].unsqueeze(1).broadcast_to([128, 2, E]),
                    r[:, 8:24].rearrange("b (j e) -> b j e", j=2),
                    op=mybir.AluOpType.mult,
                )
                ob = wpool.tile([128, 2], f32)
                nc.vector.reduce_sum(
                    ob[:], mt[:].rearrange("b (j e) -> b j e", j=2), axis=mybir.AxisListType.X
                )
                nc.vector.tensor_scalar_mul(ob[:], ob[:], pi[:])

                nc.scalar.dma_start(out_d[t * 128:(t + 1) * 128, :], ob[:])
                nc.scalar.dma_start(mask_d[t * 128:(t + 1) * 128, :], hm[:, 8:16])

                nc.tensor.matmul(
                    ploss[:, t:t + 1], hm[:], ones_col[:], start=True, stop=True
                )

            lsum_sb = cpool.tile([16, 1], f32)
            nc.vector.reduce_sum(lsum_sb[:], ploss[:], axis=mybir.AxisListType.X)
            nc.scalar.dma_start(lsum_d[:], lsum_sb[:])

    nc.compile()
    return nc


def _get_nc():
    if "nc" not in _cache:
        _cache["nc"] = _build()
    return _cache["nc"]


def kernel(x, noise, gw, gb, ew, eb):
    from concourse.bass_utils import run_bass_kernel_spmd

    nc = _get_nc()

    x = np.ascontiguousarray(np.asarray(x, dtype=np.float32)).reshape(B, L)
    noise = np.ascontiguousarray(np.asarray(noise, dtype=np.float32)).reshape(B, 1)
    gw = np.ascontiguousarray(np.asarray(gw, dtype=np.float32))
    gb = np.ascontiguousarray(np.asarray(gb, dtype=np.float32))
    ew = np.ascontiguousarray(np.asarray(ew, dtype=np.float32))
    eb = np.ascontiguousarray(np.asarray(eb, dtype=np.float32))

    ident = np.eye(128, dtype=np.float32)
    # perm16[q, n] = 1 iff q = e*2+h and n = h*8+e  (reorders (e,h) -> (h,e))
    perm16 = np.zeros((16, 16), dtype=np.float32)
    for e in range(E):
        for h in range(2):
            perm16[e * 2 + h, h * 8 + e] = 1.0
    wrow17 = np.concatenate(
        [np.ones(8, dtype=np.float32), np.zeros(16, dtype=np.float32)]
    ).reshape(1, 24)

    in_maps = []
    for c in range(NCORES):
        sl = slice(c * TOK, (c + 1) * TOK)
        in_maps.append(
            {
                "x": x[sl],
                "noise": noise[sl],
                "gw": gw,
                "gb": gb,
                "ew": ew,
                "eb": eb,
                "ident": ident,
                "perm16": perm16,
                "wrow17": wrow17,
            }
        )

    res = run_bass_kernel_spmd(nc, in_maps, core_ids=list(range(NCORES)))

    output = np.concatenate([res.results[c]["out"] for c in range(NCORES)], axis=0)
    mask = np.concatenate([res.results[c]["masko"] for c in range(NCORES)], axis=0)
    ls = np.sum([res.results[c]["lsums"][:, 0] for c in range(NCORES)], axis=0)
    h_mean = (ls[0:8] / np.float32(B)).astype(np.float32)
    m_mean = (ls[8:16] / np.float32(B)).astype(np.float32)
    loss = np.float32((h_mean * m_mean).mean() * (E * E))
    return output, mask, loss
